# revision 26
# baseline (speedup 1.0000x reference)
"""Trainium2 Bass kernel for masked attention with additive positional bias.

reference:
    scale  = 1/sqrt(2*D)
    scores = (Q K^T + pos_attn) * scale ; masked (mask==0 -> -1e9)
    p_attn = softmax(scores, axis=-1)
    out    = p_attn @ V
    returns (out, p_attn)

Shapes: Q/K/V [B=2, H=8, S=2048, D=64] f32, pos_attn [B,H,S,S] f32,
mask [B,1,1,S] int32.  B*H = 16 slabs sharded 2-per-core over 8 cores
(cores 0-3 carry batch 0, cores 4-7 batch 1).

Device-side design (per core: 2 slabs x 16 q-tiles of [128, 2048]):
  - host packs qT = [Q^T; ones] and kT = [K^T; madd] (65 x S, fp16) so the
    additive mask rides the matmul's contraction row for free; fp16 keeps
    ~2^-11 relative precision at bf16 speed on the PE
  - QK^T: fp16 matmuls into PSUM
  - DVE adds the fp32 pos_attn tile (the only tensor_tensor pass)
  - ACT computes E = exp(S * scale) f32 with accum_out giving row sums free
  - DVE reciprocal + tensor_scalar_mul -> normalized fp32 p_attn -> HBM
  - PE transposes E (128x128 f32) into PSUM; ACT copy-casts to fp16 E^T
  - PV: V (stationary fp16) x E^T -> out^T f32 accumulated over 16 k-chunks;
    small PE transpose fixup + per-partition reciprocal scale -> out

No row-max subtraction is needed: scores are bounded (~|s|<6 after scale,
masked lanes ~-2650 -> exp underflows to exactly 0.0, matching jax).
"""

import os
import sys

for _p in ("/opt/trn_rl_repo", "/root/.axon_site/_ro/trn_rl_repo"):
    if os.path.isdir(_p) and _p not in sys.path:
        sys.path.insert(0, _p)

from contextlib import ExitStack

import numpy as np

import concourse.bass as bass  # noqa: F401
import concourse.mybir as mybir
import concourse.tile as tile
from concourse import bacc
from concourse.bass_utils import run_bass_kernel_spmd
from concourse.masks import make_identity

B, H, S, D = 2, 8, 2048, 64
N_CORES = 8
SLABS_PER_CORE = (B * H) // N_CORES  # 2
SCALE = float(1.0 / np.sqrt(2 * D))  # 1/sqrt(128)
MASK_BIG = -30000.0  # * SCALE ~ -2652 -> exp == 0.0 exactly in f32
QT = S // 128  # 16 q-tiles per slab
KC = S // 128  # 16 k-chunks per slab
F16 = mybir.dt.float16
F32 = mybir.dt.float32
CROWS = D + 1  # contraction rows: 64 data + 1 mask/ones row

# --- tunables (A/B via analyze.py) ---
OPT = dict(
    store_engine="sync",  # "sync" | "scalar": HWDGE ring for p stores
    pos_pair=False,       # load pos for a q-tile pair in one 2MB DMA
    sc_width=512,         # QK psum tile width (512 -> 4 chunks, 1024 -> 2)
    pos_bufs=3,
    e_bufs=3,
    p_bufs=2,
    s_bufs=2,
    et_bufs=2,
    sc_bufs=2,
    dma_only=False,       # calibration: only pos loads + p stores
    io16=False,           # fp16 pos_attn reads + fp16 unnormalized-E stores
    pv_group=2,           # q-tiles per PV matmul group (2 or 4)
    ot_bufs=2,
    of_bufs=2,
    etcopy="scalar",      # "scalar" | "vector" | "split": engine for E^T copies
)
if os.environ.get("KERNEL_OPT"):
    import json as _json
    OPT.update(_json.loads(os.environ["KERNEL_OPT"]))


def _build_program(repeats: int = 1, loop_repeats: int = 1, opt: dict | None = None):
    cfg = dict(OPT)
    if opt:
        cfg.update(opt)
    nc = bacc.Bacc(
        "TRN2",
        debug=False,
        enable_asserts=False,
        target_bir_lowering=False,
        num_devices=N_CORES,
    )
    io_dt = F16 if cfg["io16"] else F32
    qT = nc.dram_tensor("qT", [SLABS_PER_CORE, CROWS, S], F16, kind="ExternalInput").ap()
    kT = nc.dram_tensor("kT", [SLABS_PER_CORE, CROWS, S], F16, kind="ExternalInput").ap()
    v = nc.dram_tensor("v", [SLABS_PER_CORE, S, D], F16, kind="ExternalInput").ap()
    pos = nc.dram_tensor("pos", [SLABS_PER_CORE, S, S], io_dt, kind="ExternalInput").ap()
    p = nc.dram_tensor("p", [SLABS_PER_CORE, S, S], io_dt, kind="ExternalOutput").ap()
    o = nc.dram_tensor("o", [SLABS_PER_CORE, S, D], F32, kind="ExternalOutput").ap()
    rs = None
    if cfg["io16"]:
        rs = nc.dram_tensor(
            "rs", [SLABS_PER_CORE, S], F32, kind="ExternalOutput"
        ).ap()

    with tile.TileContext(nc) as tc, ExitStack() as ctx:
        pools = dict(
            const=ctx.enter_context(tc.tile_pool(name="const", bufs=1)),
            qk=ctx.enter_context(tc.tile_pool(name="qk", bufs=2)),
            v=ctx.enter_context(tc.tile_pool(name="vp", bufs=2)),
            pos=ctx.enter_context(tc.tile_pool(name="pospool", bufs=cfg["pos_bufs"])),
            s=ctx.enter_context(tc.tile_pool(name="spool", bufs=cfg["s_bufs"])),
            e=ctx.enter_context(tc.tile_pool(name="epool", bufs=cfg["e_bufs"])),
            p=ctx.enter_context(tc.tile_pool(name="ppool", bufs=cfg["p_bufs"])),
            et=ctx.enter_context(tc.tile_pool(name="etpool", bufs=cfg["et_bufs"])),
            stat=ctx.enter_context(tc.tile_pool(name="stat", bufs=8)),
            out=ctx.enter_context(tc.tile_pool(name="outp", bufs=2)),
            ps_sc=ctx.enter_context(
                tc.tile_pool(name="ps_sc", bufs=cfg["sc_bufs"], space="PSUM")
            ),
            ps_et=ctx.enter_context(tc.tile_pool(name="ps_et", bufs=2, space="PSUM")),
            ps_ot=ctx.enter_context(
                tc.tile_pool(name="ps_ot", bufs=cfg["ot_bufs"], space="PSUM")
            ),
            ps_of=ctx.enter_context(
                tc.tile_pool(name="ps_of", bufs=cfg["of_bufs"], space="PSUM")
            ),
        )

        ident = pools["const"].tile([128, 128], F32)
        make_identity(nc, ident)
        ident16 = None
        if cfg["io16"]:
            ident16 = pools["const"].tile([128, 128], F16)
            make_identity(nc, ident16)

        loop_cm = tc.For_i(0, loop_repeats, 1) if loop_repeats > 1 else ExitStack()
        with loop_cm:
            _emit_body(
                nc, cfg, repeats, qT, kT, v, pos, p, o, rs, pools, ident, ident16
            )

    nc.finalize()
    return nc


def _emit_body(nc, cfg, repeats, qT, kT, v, pos, p, o, rs, pools, ident, ident16):
    store_eng = {
        "scalar": nc.scalar, "gpsimd": nc.gpsimd, "sync": nc.sync
    }[cfg["store_engine"]]
    scw = cfg["sc_width"]
    n_sc = S // scw

    if cfg["dma_only"]:
        # calibration: stream pos in and write it back out as p
        for s in [s for _ in range(repeats) for s in range(SLABS_PER_CORE)]:
            ob = pools["out"].tile([128, QT, D], F32, tag="outslab")
            nc.vector.memset(ob[:, 0, :], 0.0)
            nc.sync.dma_start(out=o[s].rearrange("(qt p) d -> p qt d", p=128), in_=ob)
            for qt in range(QT):
                q0 = qt * 128
                pos_t = pools["pos"].tile([128, S], F32, tag="pos")
                nc.sync.dma_start(out=pos_t, in_=pos[s, q0 : q0 + 128, :])
                store_eng.dma_start(out=p[s, q0 : q0 + 128, :], in_=pos_t)
        return

    io16 = cfg["io16"]
    e_dt = F16 if io16 else F32

    for s in [s for _ in range(repeats) for s in range(SLABS_PER_CORE)]:
        qT_sb = pools["qk"].tile([CROWS, S], F16, tag="qT")
        nc.sync.dma_start(out=qT_sb, in_=qT[s])
        kT_sb = pools["qk"].tile([CROWS, S], F16, tag="kT")
        nc.sync.dma_start(out=kT_sb, in_=kT[s])
        v_sb = pools["v"].tile([128, KC, D], F16, tag="v")
        nc.sync.dma_start(out=v_sb, in_=v[s].rearrange("(kc p) d -> p kc d", p=128))
        out_slab = pools["out"].tile([128, QT, D], F32, tag="outslab")
        rs_slab = None
        if io16:
            rs_slab = pools["out"].tile([128, QT], F32, tag="rsslab")

        G = cfg["pv_group"]
        for qg in range(QT // G):
            # E^T staging for the group's q-tiles, fp16,
            # laid out [k_local(128 part), j(G), kc(16), q_local(128)]
            et_sb = pools["et"].tile([128, G, KC, 128], F16, tag="et")
            recips = []
            pos_pair_t = None
            if cfg["pos_pair"]:
                q0p = qg * G * 128
                pos_pair_t = pools["pos"].tile([128, G, S], e_dt if io16 else F32, tag="pos")
                nc.sync.dma_start(
                    out=pos_pair_t,
                    in_=pos[s, q0p : q0p + G * 128, :].rearrange(
                        "(j p) m -> p j m", p=128
                    ),
                )
            for j in range(G):
                qt = qg * G + j
                q0 = qt * 128
                if cfg["pos_pair"]:
                    pos_t = pos_pair_t[:, j, :]
                else:
                    pos_t = pools["pos"].tile([128, S], e_dt if io16 else F32, tag="pos")
                    nc.sync.dma_start(out=pos_t, in_=pos[s, q0 : q0 + 128, :])

                s_sb = pools["s"].tile([128, S], F32, tag="s")
                for c in range(n_sc):
                    sc_ps = pools["ps_sc"].tile([128, scw], F32, tag="sc")
                    for cc in range(scw // 512):
                        nc.tensor.matmul(
                            sc_ps[:, cc * 512 : (cc + 1) * 512],
                            lhsT=qT_sb[:, q0 : q0 + 128],
                            rhs=kT_sb[:, c * scw + cc * 512 : c * scw + (cc + 1) * 512],
                            start=True,
                            stop=True,
                        )
                    nc.vector.tensor_add(
                        s_sb[:, c * scw : (c + 1) * scw],
                        sc_ps,
                        pos_t[:, c * scw : (c + 1) * scw],
                    )

                e_sb = pools["e"].tile([128, S], e_dt, tag="e")
                rowsum = (
                    rs_slab[:, qt : qt + 1]
                    if io16
                    else pools["stat"].tile([128, 1], F32, tag="rowsum")
                )
                nc.scalar.activation(
                    e_sb,
                    s_sb,
                    mybir.ActivationFunctionType.Exp,
                    bias=0.0,
                    scale=SCALE,
                    accum_out=rowsum,
                )
                recip = pools["stat"].tile([128, 1], F32, tag="recip")
                nc.vector.reciprocal(recip, rowsum)
                recips.append(recip)

                if io16:
                    # store unnormalized E (fp16); host divides by rowsum
                    store_eng.dma_start(out=p[s, q0 : q0 + 128, :], in_=e_sb)
                    # transpose E 128x128 fp16 tiles via PE, 8 per PSUM bank
                    for c in range(2):
                        et_ps = pools["ps_et"].tile([128, 8, 128], F16, tag="etps")
                        for jj in range(8):
                            kc = c * 8 + jj
                            nc.tensor.transpose(
                                et_ps[:, jj, :],
                                e_sb[:, kc * 128 : (kc + 1) * 128],
                                ident16,
                            )
                        dst = et_sb[:, j, c * 8 : (c + 1) * 8, :]
                        if cfg["etcopy"] == "vector" or (
                            cfg["etcopy"] == "split" and c == 1
                        ):
                            nc.vector.tensor_copy(dst, et_ps)
                        else:
                            nc.scalar.copy(dst, et_ps)
                else:
                    p_sb = pools["p"].tile([128, S], F32, tag="p")
                    nc.vector.tensor_scalar_mul(p_sb, e_sb, recip)
                    store_eng.dma_start(out=p[s, q0 : q0 + 128, :], in_=p_sb)

                    # transpose E 128x128 tiles via PE, 4 per PSUM bank,
                    # then one ACT copy-cast f32->fp16 per bank
                    for c in range(4):
                        et_ps = pools["ps_et"].tile([128, 4, 128], F32, tag="etps")
                        for jj in range(4):
                            kc = c * 4 + jj
                            nc.tensor.transpose(
                                et_ps[:, jj, :],
                                e_sb[:, kc * 128 : (kc + 1) * 128],
                                ident,
                            )
                        nc.scalar.copy(et_sb[:, j, c * 4 : (c + 1) * 4, :], et_ps)

            # PV for the group: out^T[d, (j, q_local)] accumulated over kc
            ot_ps = pools["ps_ot"].tile([D, G, 128], F32, tag="ot")
            for kc in range(KC):
                nc.tensor.matmul(
                    ot_ps,
                    lhsT=v_sb[:, kc, :],
                    rhs=et_sb[:, :, kc, :],
                    start=(kc == 0),
                    stop=(kc == KC - 1),
                )
            ot_sb = pools["stat"].tile([D, G, 128], F32, tag="ot_sb")
            nc.vector.tensor_copy(ot_sb, ot_ps)
            for j in range(G):
                qt = qg * G + j
                of_ps = pools["ps_of"].tile([128, D], F32, tag="of")
                nc.tensor.transpose(of_ps, ot_sb[:, j, :], ident[:D, :D])
                nc.vector.tensor_scalar_mul(out_slab[:, qt, :], of_ps, recips[j])

        nc.sync.dma_start(
            out=o[s].rearrange("(qt p) d -> p qt d", p=128), in_=out_slab
        )
        if io16:
            nc.sync.dma_start(
                out=rs[s].rearrange("(qt p) -> p qt", p=128), in_=rs_slab
            )


_NC = None


def _get_program():
    global _NC
    if _NC is None:
        _NC = _build_program()
    return _NC


def _prep_inputs(query, key, value, pos_attn, mask):
    """Host-side shard + pack: per-core input maps."""
    query = np.asarray(query, dtype=np.float32)
    key_ = np.asarray(key, dtype=np.float32)
    value = np.asarray(value, dtype=np.float32)
    pos_attn = np.asarray(pos_attn)
    mask = np.asarray(mask)

    # madd[b, k]: 0 where mask==1 else MASK_BIG (exact in fp16)
    madd = np.where(mask[:, 0, 0, :] == 0, np.float32(MASK_BIG), np.float32(0.0))

    pos_dt = np.float16 if OPT["io16"] else np.float32
    in_maps = []
    for core in range(N_CORES):
        qT_arr = np.zeros((SLABS_PER_CORE, CROWS, S), dtype=np.float16)
        kT_arr = np.zeros((SLABS_PER_CORE, CROWS, S), dtype=np.float16)
        v_arr = np.zeros((SLABS_PER_CORE, S, D), dtype=np.float16)
        pos_arr = np.empty((SLABS_PER_CORE, S, S), dtype=pos_dt)
        for s in range(SLABS_PER_CORE):
            slab = core * SLABS_PER_CORE + s
            b, h = slab // H, slab % H
            qT_arr[s, :D, :] = query[b, h].T.astype(np.float16)
            qT_arr[s, D, :] = np.float16(1.0)
            kT_arr[s, :D, :] = key_[b, h].T.astype(np.float16)
            kT_arr[s, D, :] = madd[b].astype(np.float16)
            v_arr[s] = value[b, h].astype(np.float16)
            pos_arr[s] = np.asarray(pos_attn[b, h], dtype=pos_dt)
        in_maps.append({"qT": qT_arr, "kT": kT_arr, "v": v_arr, "pos": pos_arr})
    return in_maps


LAST_RESULTS = None


def kernel(query, key, value, pos_attn, mask, **run_kwargs):
    global LAST_RESULTS
    nc = _get_program()
    in_maps = _prep_inputs(query, key, value, pos_attn, mask)
    res = run_bass_kernel_spmd(
        nc, in_maps, core_ids=list(range(N_CORES)), **run_kwargs
    )
    LAST_RESULTS = res

    out = np.empty((B, H, S, D), dtype=np.float32)
    p_attn = np.empty((B, H, S, S), dtype=np.float32)
    for core in range(N_CORES):
        rm = res.results[core]
        for s in range(SLABS_PER_CORE):
            slab = core * SLABS_PER_CORE + s
            b, h = slab // H, slab % H
            out[b, h] = rm["o"][s]
            if OPT["io16"]:
                recip = (1.0 / rm["rs"][s]).astype(np.float32)
                np.multiply(rm["p"][s], recip[:, None], out=p_attn[b, h])
            else:
                p_attn[b, h] = rm["p"][s]
    return out, p_attn


# revision 28
# speedup vs baseline: 1.1770x; 1.1770x over previous
"""Trainium2 Bass kernel for masked attention with additive positional bias.

reference:
    scale  = 1/sqrt(2*D)
    scores = (Q K^T + pos_attn) * scale ; masked (mask==0 -> -1e9)
    p_attn = softmax(scores, axis=-1)
    out    = p_attn @ V
    returns (out, p_attn)

Shapes: Q/K/V [B=2, H=8, S=2048, D=64] f32, pos_attn [B,H,S,S] f32,
mask [B,1,1,S] int32.  B*H = 16 slabs sharded 2-per-core over 8 cores
(cores 0-3 carry batch 0, cores 4-7 batch 1).

Device-side design (per core: 2 slabs x 16 q-tiles of [128, 2048]):
  - host packs qT = [Q^T; ones] and kT = [K^T; madd] (65 x S, fp16) so the
    additive mask rides the matmul's contraction row for free; fp16 keeps
    ~2^-11 relative precision at bf16 speed on the PE
  - QK^T: fp16 matmuls into PSUM
  - DVE adds the fp32 pos_attn tile (the only tensor_tensor pass)
  - ACT computes E = exp(S * scale) f32 with accum_out giving row sums free
  - DVE reciprocal + tensor_scalar_mul -> normalized fp32 p_attn -> HBM
  - PE transposes E (128x128 f32) into PSUM; ACT copy-casts to fp16 E^T
  - PV: V (stationary fp16) x E^T -> out^T f32 accumulated over 16 k-chunks;
    small PE transpose fixup + per-partition reciprocal scale -> out

No row-max subtraction is needed: scores are bounded (~|s|<6 after scale,
masked lanes ~-2650 -> exp underflows to exactly 0.0, matching jax).
"""

import os
import sys

for _p in ("/opt/trn_rl_repo", "/root/.axon_site/_ro/trn_rl_repo"):
    if os.path.isdir(_p) and _p not in sys.path:
        sys.path.insert(0, _p)

from contextlib import ExitStack

import numpy as np

import concourse.bass as bass  # noqa: F401
import concourse.mybir as mybir
import concourse.tile as tile
from concourse import bacc
from concourse.bass_utils import run_bass_kernel_spmd
from concourse.masks import make_identity

B, H, S, D = 2, 8, 2048, 64
N_CORES = 8
SLABS_PER_CORE = (B * H) // N_CORES  # 2
SCALE = float(1.0 / np.sqrt(2 * D))  # 1/sqrt(128)
MASK_BIG = -30000.0  # * SCALE ~ -2652 -> exp == 0.0 exactly in f32
QT = S // 128  # 16 q-tiles per slab
KC = S // 128  # 16 k-chunks per slab
F16 = mybir.dt.float16
F32 = mybir.dt.float32
CROWS = D + 1  # contraction rows: 64 data + 1 mask/ones row

# --- tunables (A/B via analyze.py) ---
OPT = dict(
    store_engine="sync",  # "sync" | "scalar": HWDGE ring for p stores
    pos_pair=False,       # load pos for a q-tile pair in one 2MB DMA
    sc_width=512,         # QK psum tile width (512 -> 4 chunks, 1024 -> 2)
    pos_bufs=3,
    e_bufs=3,
    p_bufs=2,
    s_bufs=2,
    et_bufs=2,
    sc_bufs=2,
    dma_only=False,       # calibration: only pos loads + p stores
    io16=False,           # fp16 pos_attn reads + fp16 unnormalized-E stores
    pv_group=2,           # q-tiles per PV matmul group (2 or 4)
    ot_bufs=2,
    of_bufs=2,
    etcopy="scalar",      # "scalar" | "vector" | "split": engine for E^T copies
    skip_store=False,     # ablation probe: skip p stores
    skip_pv=False,        # ablation probe: skip transposes/PV/out path
)
if os.environ.get("KERNEL_OPT"):
    import json as _json
    OPT.update(_json.loads(os.environ["KERNEL_OPT"]))


def _build_program(repeats: int = 1, loop_repeats: int = 1, opt: dict | None = None):
    cfg = dict(OPT)
    if opt:
        cfg.update(opt)
    nc = bacc.Bacc(
        "TRN2",
        debug=False,
        enable_asserts=False,
        target_bir_lowering=False,
        num_devices=N_CORES,
    )
    io_dt = F16 if cfg["io16"] else F32
    qT = nc.dram_tensor("qT", [SLABS_PER_CORE, CROWS, S], F16, kind="ExternalInput").ap()
    kT = nc.dram_tensor("kT", [SLABS_PER_CORE, CROWS, S], F16, kind="ExternalInput").ap()
    v = nc.dram_tensor("v", [SLABS_PER_CORE, S, D], F16, kind="ExternalInput").ap()
    pos = nc.dram_tensor("pos", [SLABS_PER_CORE, S, S], io_dt, kind="ExternalInput").ap()
    p = nc.dram_tensor("p", [SLABS_PER_CORE, S, S], io_dt, kind="ExternalOutput").ap()
    o = nc.dram_tensor("o", [SLABS_PER_CORE, S, D], F32, kind="ExternalOutput").ap()
    rs = None
    if cfg["io16"]:
        rs = nc.dram_tensor(
            "rs", [SLABS_PER_CORE, S], F32, kind="ExternalOutput"
        ).ap()

    with tile.TileContext(nc) as tc, ExitStack() as ctx:
        pools = dict(
            const=ctx.enter_context(tc.tile_pool(name="const", bufs=1)),
            qk=ctx.enter_context(tc.tile_pool(name="qk", bufs=2)),
            v=ctx.enter_context(tc.tile_pool(name="vp", bufs=2)),
            pos=ctx.enter_context(tc.tile_pool(name="pospool", bufs=cfg["pos_bufs"])),
            s=ctx.enter_context(tc.tile_pool(name="spool", bufs=cfg["s_bufs"])),
            e=ctx.enter_context(tc.tile_pool(name="epool", bufs=cfg["e_bufs"])),
            p=ctx.enter_context(tc.tile_pool(name="ppool", bufs=cfg["p_bufs"])),
            et=ctx.enter_context(tc.tile_pool(name="etpool", bufs=cfg["et_bufs"])),
            stat=ctx.enter_context(tc.tile_pool(name="stat", bufs=8)),
            out=ctx.enter_context(tc.tile_pool(name="outp", bufs=2)),
            ps_sc=ctx.enter_context(
                tc.tile_pool(name="ps_sc", bufs=cfg["sc_bufs"], space="PSUM")
            ),
            ps_et=ctx.enter_context(tc.tile_pool(name="ps_et", bufs=2, space="PSUM")),
            ps_ot=ctx.enter_context(
                tc.tile_pool(name="ps_ot", bufs=cfg["ot_bufs"], space="PSUM")
            ),
            ps_of=ctx.enter_context(
                tc.tile_pool(name="ps_of", bufs=cfg["of_bufs"], space="PSUM")
            ),
        )

        ident = pools["const"].tile([128, 128], F32)
        make_identity(nc, ident)
        ident16 = None
        if cfg["io16"]:
            ident16 = pools["const"].tile([128, 128], F16)
            make_identity(nc, ident16)

        loop_cm = tc.For_i(0, loop_repeats, 1) if loop_repeats > 1 else ExitStack()
        with loop_cm:
            _emit_body(
                nc, cfg, repeats, qT, kT, v, pos, p, o, rs, pools, ident, ident16
            )

    nc.finalize()
    return nc


def _emit_body(nc, cfg, repeats, qT, kT, v, pos, p, o, rs, pools, ident, ident16):
    store_eng = {
        "scalar": nc.scalar, "gpsimd": nc.gpsimd, "sync": nc.sync
    }[cfg["store_engine"]]
    scw = cfg["sc_width"]
    n_sc = S // scw

    if cfg["dma_only"]:
        # calibration: stream pos in and write it back out as p
        for s in [s for _ in range(repeats) for s in range(SLABS_PER_CORE)]:
            ob = pools["out"].tile([128, QT, D], F32, tag="outslab")
            nc.vector.memset(ob[:, 0, :], 0.0)
            nc.sync.dma_start(out=o[s].rearrange("(qt p) d -> p qt d", p=128), in_=ob)
            for qt in range(QT):
                q0 = qt * 128
                pos_t = pools["pos"].tile([128, S], F32, tag="pos")
                nc.sync.dma_start(out=pos_t, in_=pos[s, q0 : q0 + 128, :])
                store_eng.dma_start(out=p[s, q0 : q0 + 128, :], in_=pos_t)
        return

    io16 = cfg["io16"]
    e_dt = F16 if io16 else F32

    for s in [s for _ in range(repeats) for s in range(SLABS_PER_CORE)]:
        qT_sb = pools["qk"].tile([CROWS, S], F16, tag="qT")
        nc.sync.dma_start(out=qT_sb, in_=qT[s])
        kT_sb = pools["qk"].tile([CROWS, S], F16, tag="kT")
        nc.sync.dma_start(out=kT_sb, in_=kT[s])
        v_sb = pools["v"].tile([128, KC, D], F16, tag="v")
        nc.sync.dma_start(out=v_sb, in_=v[s].rearrange("(kc p) d -> p kc d", p=128))
        out_slab = pools["out"].tile([128, QT, D], F32, tag="outslab")
        if cfg["skip_pv"]:
            nc.vector.memset(out_slab[:, 0, :], 0.0)
        rs_slab = None
        if io16:
            rs_slab = pools["out"].tile([128, QT], F32, tag="rsslab")

        G = cfg["pv_group"]
        for qg in range(QT // G):
            # E^T staging for the group's q-tiles, fp16,
            # laid out [k_local(128 part), j(G), kc(16), q_local(128)]
            et_sb = pools["et"].tile([128, G, KC, 128], F16, tag="et")
            recips = []
            pos_pair_t = None
            if cfg["pos_pair"]:
                q0p = qg * G * 128
                pos_pair_t = pools["pos"].tile([128, G, S], e_dt if io16 else F32, tag="pos")
                nc.sync.dma_start(
                    out=pos_pair_t,
                    in_=pos[s, q0p : q0p + G * 128, :].rearrange(
                        "(j p) m -> p j m", p=128
                    ),
                )
            for j in range(G):
                qt = qg * G + j
                q0 = qt * 128
                if cfg["pos_pair"]:
                    pos_t = pos_pair_t[:, j, :]
                else:
                    pos_t = pools["pos"].tile([128, S], e_dt if io16 else F32, tag="pos")
                    nc.sync.dma_start(out=pos_t, in_=pos[s, q0 : q0 + 128, :])

                s_sb = pools["s"].tile([128, S], F32, tag="s")
                for c in range(n_sc):
                    sc_ps = pools["ps_sc"].tile([128, scw], F32, tag="sc")
                    for cc in range(scw // 512):
                        nc.tensor.matmul(
                            sc_ps[:, cc * 512 : (cc + 1) * 512],
                            lhsT=qT_sb[:, q0 : q0 + 128],
                            rhs=kT_sb[:, c * scw + cc * 512 : c * scw + (cc + 1) * 512],
                            start=True,
                            stop=True,
                        )
                    nc.vector.tensor_add(
                        s_sb[:, c * scw : (c + 1) * scw],
                        sc_ps,
                        pos_t[:, c * scw : (c + 1) * scw],
                    )

                e_sb = pools["e"].tile([128, S], e_dt, tag="e")
                rowsum = (
                    rs_slab[:, qt : qt + 1]
                    if io16
                    else pools["stat"].tile([128, 1], F32, tag="rowsum")
                )
                nc.scalar.activation(
                    e_sb,
                    s_sb,
                    mybir.ActivationFunctionType.Exp,
                    bias=0.0,
                    scale=SCALE,
                    accum_out=rowsum,
                )
                recip = pools["stat"].tile([128, 1], F32, tag="recip")
                nc.vector.reciprocal(recip, rowsum)
                recips.append(recip)

                if io16:
                    # store unnormalized E (fp16); host divides by rowsum
                    if not cfg["skip_store"]:
                        store_eng.dma_start(out=p[s, q0 : q0 + 128, :], in_=e_sb)
                    if cfg["skip_pv"]:
                        continue
                    # transpose E 128x128 fp16 tiles via PE, 8 per PSUM bank
                    for c in range(2):
                        et_ps = pools["ps_et"].tile([128, 8, 128], F16, tag="etps")
                        for jj in range(8):
                            kc = c * 8 + jj
                            nc.tensor.transpose(
                                et_ps[:, jj, :],
                                e_sb[:, kc * 128 : (kc + 1) * 128],
                                ident16,
                            )
                        dst = et_sb[:, j, c * 8 : (c + 1) * 8, :]
                        if cfg["etcopy"] == "vector" or (
                            cfg["etcopy"] == "split" and c == 1
                        ):
                            nc.vector.tensor_copy(dst, et_ps)
                        else:
                            nc.scalar.copy(dst, et_ps)
                else:
                    p_sb = pools["p"].tile([128, S], F32, tag="p")
                    nc.vector.tensor_scalar_mul(p_sb, e_sb, recip)
                    store_eng.dma_start(out=p[s, q0 : q0 + 128, :], in_=p_sb)

                    # transpose E 128x128 tiles via PE, 4 per PSUM bank,
                    # then one ACT copy-cast f32->fp16 per bank
                    for c in range(4):
                        et_ps = pools["ps_et"].tile([128, 4, 128], F32, tag="etps")
                        for jj in range(4):
                            kc = c * 4 + jj
                            nc.tensor.transpose(
                                et_ps[:, jj, :],
                                e_sb[:, kc * 128 : (kc + 1) * 128],
                                ident,
                            )
                        nc.scalar.copy(et_sb[:, j, c * 4 : (c + 1) * 4, :], et_ps)

            if cfg["skip_pv"]:
                continue
            # PV for the group: out^T[d, (j, q_local)] accumulated over kc
            ot_ps = pools["ps_ot"].tile([D, G, 128], F32, tag="ot")
            for kc in range(KC):
                nc.tensor.matmul(
                    ot_ps,
                    lhsT=v_sb[:, kc, :],
                    rhs=et_sb[:, :, kc, :],
                    start=(kc == 0),
                    stop=(kc == KC - 1),
                )
            ot_sb = pools["stat"].tile([D, G, 128], F32, tag="ot_sb")
            nc.vector.tensor_copy(ot_sb, ot_ps)
            for j in range(G):
                qt = qg * G + j
                of_ps = pools["ps_of"].tile([128, D], F32, tag="of")
                nc.tensor.transpose(of_ps, ot_sb[:, j, :], ident[:D, :D])
                nc.vector.tensor_scalar_mul(out_slab[:, qt, :], of_ps, recips[j])

        nc.sync.dma_start(
            out=o[s].rearrange("(qt p) d -> p qt d", p=128), in_=out_slab
        )
        if io16:
            nc.sync.dma_start(
                out=rs[s].rearrange("(qt p) -> p qt", p=128), in_=rs_slab
            )


_NC = None


def _get_program():
    global _NC
    if _NC is None:
        _NC = _build_program()
    return _NC


def _prep_inputs(query, key, value, pos_attn, mask):
    """Host-side shard + pack: per-core input maps."""
    query = np.asarray(query, dtype=np.float32)
    key_ = np.asarray(key, dtype=np.float32)
    value = np.asarray(value, dtype=np.float32)
    pos_attn = np.asarray(pos_attn)
    mask = np.asarray(mask)

    # madd[b, k]: 0 where mask==1 else MASK_BIG (exact in fp16)
    madd = np.where(mask[:, 0, 0, :] == 0, np.float32(MASK_BIG), np.float32(0.0))

    pos_dt = np.float16 if OPT["io16"] else np.float32
    in_maps = []
    for core in range(N_CORES):
        qT_arr = np.zeros((SLABS_PER_CORE, CROWS, S), dtype=np.float16)
        kT_arr = np.zeros((SLABS_PER_CORE, CROWS, S), dtype=np.float16)
        v_arr = np.zeros((SLABS_PER_CORE, S, D), dtype=np.float16)
        pos_arr = np.empty((SLABS_PER_CORE, S, S), dtype=pos_dt)
        for s in range(SLABS_PER_CORE):
            slab = core * SLABS_PER_CORE + s
            b, h = slab // H, slab % H
            qT_arr[s, :D, :] = query[b, h].T.astype(np.float16)
            qT_arr[s, D, :] = np.float16(1.0)
            kT_arr[s, :D, :] = key_[b, h].T.astype(np.float16)
            kT_arr[s, D, :] = madd[b].astype(np.float16)
            v_arr[s] = value[b, h].astype(np.float16)
            pos_arr[s] = np.asarray(pos_attn[b, h], dtype=pos_dt)
        in_maps.append({"qT": qT_arr, "kT": kT_arr, "v": v_arr, "pos": pos_arr})
    return in_maps


LAST_RESULTS = None


def kernel(query, key, value, pos_attn, mask, **run_kwargs):
    global LAST_RESULTS
    nc = _get_program()
    in_maps = _prep_inputs(query, key, value, pos_attn, mask)
    res = run_bass_kernel_spmd(
        nc, in_maps, core_ids=list(range(N_CORES)), **run_kwargs
    )
    LAST_RESULTS = res

    out = np.empty((B, H, S, D), dtype=np.float32)
    p_attn = np.empty((B, H, S, S), dtype=np.float32)
    for core in range(N_CORES):
        rm = res.results[core]
        for s in range(SLABS_PER_CORE):
            slab = core * SLABS_PER_CORE + s
            b, h = slab // H, slab % H
            out[b, h] = rm["o"][s]
            if OPT["io16"]:
                recip = (1.0 / rm["rs"][s]).astype(np.float32)
                np.multiply(rm["p"][s], recip[:, None], out=p_attn[b, h])
            else:
                p_attn[b, h] = rm["p"][s]
    return out, p_attn


# revision 31
# speedup vs baseline: 1.8669x; 1.5861x over previous
"""Trainium2 Bass kernel for masked attention with additive positional bias.

reference:
    scale  = 1/sqrt(2*D)
    scores = (Q K^T + pos_attn) * scale ; masked (mask==0 -> -1e9)
    p_attn = softmax(scores, axis=-1)
    out    = p_attn @ V
    returns (out, p_attn)

Shapes: Q/K/V [B=2, H=8, S=2048, D=64] f32, pos_attn [B,H,S,S] f32,
mask [B,1,1,S] int32.  B*H = 16 slabs sharded 2-per-core over 8 cores
(cores 0-3 carry batch 0, cores 4-7 batch 1).

Device-side design (per core: 2 slabs x 16 q-tiles of [128, 2048]):
  - host packs qT = [Q^T; ones] and kT = [K^T; madd] (65 x S, fp16) so the
    additive mask rides the matmul's contraction row for free; fp16 keeps
    ~2^-11 relative precision at bf16 speed on the PE
  - QK^T: fp16 matmuls into PSUM
  - DVE adds the fp32 pos_attn tile (the only tensor_tensor pass)
  - ACT computes E = exp(S * scale) f32 with accum_out giving row sums free
  - DVE reciprocal + tensor_scalar_mul -> normalized fp32 p_attn -> HBM
  - PE transposes E (128x128 f32) into PSUM; ACT copy-casts to fp16 E^T
  - PV: V (stationary fp16) x E^T -> out^T f32 accumulated over 16 k-chunks;
    small PE transpose fixup + per-partition reciprocal scale -> out

No row-max subtraction is needed: scores are bounded (~|s|<6 after scale,
masked lanes ~-2650 -> exp underflows to exactly 0.0, matching jax).
"""

import os
import sys

for _p in ("/opt/trn_rl_repo", "/root/.axon_site/_ro/trn_rl_repo"):
    if os.path.isdir(_p) and _p not in sys.path:
        sys.path.insert(0, _p)

from contextlib import ExitStack

import numpy as np

import concourse.bass as bass  # noqa: F401
import concourse.mybir as mybir
import concourse.tile as tile
from concourse import bacc
from concourse.bass_utils import run_bass_kernel_spmd
from concourse.masks import make_identity

B, H, S, D = 2, 8, 2048, 64
N_CORES = 8
SLABS_PER_CORE = (B * H) // N_CORES  # 2
SCALE = float(1.0 / np.sqrt(2 * D))  # 1/sqrt(128)
MASK_BIG = -30000.0  # * SCALE ~ -2652 -> exp == 0.0 exactly in f32
QT = S // 128  # 16 q-tiles per slab
KC = S // 128  # 16 k-chunks per slab
F16 = mybir.dt.float16
F32 = mybir.dt.float32
CROWS = D + 1  # contraction rows: 64 data + 1 mask/ones row

# --- tunables (A/B via analyze.py) ---
OPT = dict(
    store_engine="sync",  # "sync" | "scalar": HWDGE ring for p stores
    pos_pair=False,       # load pos for a q-tile pair in one 2MB DMA
    sc_width=512,         # QK psum tile width (512 -> 4 chunks, 1024 -> 2)
    pos_bufs=3,
    e_bufs=3,
    p_bufs=2,
    s_bufs=2,
    et_bufs=2,
    sc_bufs=2,
    dma_only=False,       # calibration: only pos loads + p stores
    io16=False,           # fp16 pos_attn reads + fp16 unnormalized-E stores
    pv_group=2,           # q-tiles per PV matmul group (2 or 4)
    ot_bufs=1,
    of_bufs=1,
    etcopy="scalar",      # "scalar" | "vector" | "split": engine for E^T copies
    skip_store=False,     # ablation probe: skip p stores
    skip_pv=False,        # ablation probe: skip transposes/PV/out path
    posadd_pe=0,          # chunks (0/2/4) whose pos-add rides PE identity-MM
    otfix_act=False,      # out fixup copy+scale on ACT instead of DVE
)
if os.environ.get("KERNEL_OPT"):
    import json as _json
    OPT.update(_json.loads(os.environ["KERNEL_OPT"]))


def _build_program(repeats: int = 1, loop_repeats: int = 1, opt: dict | None = None):
    cfg = dict(OPT)
    if opt:
        cfg.update(opt)
    nc = bacc.Bacc(
        "TRN2",
        debug=False,
        enable_asserts=False,
        target_bir_lowering=False,
        num_devices=N_CORES,
    )
    io_dt = F16 if cfg["io16"] else F32
    qT = nc.dram_tensor("qT", [SLABS_PER_CORE, CROWS, S], F16, kind="ExternalInput").ap()
    kT = nc.dram_tensor("kT", [SLABS_PER_CORE, CROWS, S], F16, kind="ExternalInput").ap()
    v = nc.dram_tensor("v", [SLABS_PER_CORE, S, D], F16, kind="ExternalInput").ap()
    pos = nc.dram_tensor("pos", [SLABS_PER_CORE, S, S], io_dt, kind="ExternalInput").ap()
    p = nc.dram_tensor("p", [SLABS_PER_CORE, S, S], io_dt, kind="ExternalOutput").ap()
    o = nc.dram_tensor("o", [SLABS_PER_CORE, S, D], F32, kind="ExternalOutput").ap()
    rs = None
    if cfg["io16"]:
        rs = nc.dram_tensor(
            "rs", [SLABS_PER_CORE, S], F32, kind="ExternalOutput"
        ).ap()

    with tile.TileContext(nc) as tc, ExitStack() as ctx:
        pools = dict(
            const=ctx.enter_context(tc.tile_pool(name="const", bufs=1)),
            qk=ctx.enter_context(tc.tile_pool(name="qk", bufs=2)),
            v=ctx.enter_context(tc.tile_pool(name="vp", bufs=2)),
            pos=ctx.enter_context(tc.tile_pool(name="pospool", bufs=cfg["pos_bufs"])),
            s=ctx.enter_context(tc.tile_pool(name="spool", bufs=cfg["s_bufs"])),
            e=ctx.enter_context(tc.tile_pool(name="epool", bufs=cfg["e_bufs"])),
            p=ctx.enter_context(tc.tile_pool(name="ppool", bufs=cfg["p_bufs"])),
            et=ctx.enter_context(tc.tile_pool(name="etpool", bufs=cfg["et_bufs"])),
            stat=ctx.enter_context(tc.tile_pool(name="stat", bufs=8)),
            out=ctx.enter_context(tc.tile_pool(name="outp", bufs=2)),
            ps_sc=ctx.enter_context(
                tc.tile_pool(name="ps_sc", bufs=cfg["sc_bufs"], space="PSUM")
            ),
            ps_et=ctx.enter_context(tc.tile_pool(name="ps_et", bufs=2, space="PSUM")),
            ps_ot=ctx.enter_context(
                tc.tile_pool(name="ps_ot", bufs=cfg["ot_bufs"], space="PSUM")
            ),
            ps_of=ctx.enter_context(
                tc.tile_pool(name="ps_of", bufs=cfg["of_bufs"], space="PSUM")
            ),
        )

        ident = pools["const"].tile([128, 128], F32)
        make_identity(nc, ident)
        ident16 = None
        if cfg["io16"]:
            ident16 = pools["const"].tile([128, 128], F16)
            make_identity(nc, ident16)

        loop_cm = tc.For_i(0, loop_repeats, 1) if loop_repeats > 1 else ExitStack()
        with loop_cm:
            _emit_body(
                nc, cfg, repeats, qT, kT, v, pos, p, o, rs, pools, ident, ident16
            )

    nc.finalize()
    return nc


def _emit_body(nc, cfg, repeats, qT, kT, v, pos, p, o, rs, pools, ident, ident16):
    store_eng = {
        "scalar": nc.scalar, "gpsimd": nc.gpsimd, "sync": nc.sync
    }[cfg["store_engine"]]
    scw = cfg["sc_width"]
    n_sc = S // scw

    if cfg["dma_only"]:
        # calibration: stream pos in and write it back out as p
        for s in [s for _ in range(repeats) for s in range(SLABS_PER_CORE)]:
            ob = pools["out"].tile([128, QT, D], F32, tag="outslab")
            nc.vector.memset(ob[:, 0, :], 0.0)
            nc.sync.dma_start(out=o[s].rearrange("(qt p) d -> p qt d", p=128), in_=ob)
            for qt in range(QT):
                q0 = qt * 128
                pos_t = pools["pos"].tile([128, S], F32, tag="pos")
                nc.sync.dma_start(out=pos_t, in_=pos[s, q0 : q0 + 128, :])
                store_eng.dma_start(out=p[s, q0 : q0 + 128, :], in_=pos_t)
        return

    io16 = cfg["io16"]
    e_dt = F16 if io16 else F32

    for s in [s for _ in range(repeats) for s in range(SLABS_PER_CORE)]:
        qT_sb = pools["qk"].tile([CROWS, S], F16, tag="qT")
        nc.sync.dma_start(out=qT_sb, in_=qT[s])
        kT_sb = pools["qk"].tile([CROWS, S], F16, tag="kT")
        nc.sync.dma_start(out=kT_sb, in_=kT[s])
        v_sb = pools["v"].tile([128, KC, D], F16, tag="v")
        nc.sync.dma_start(out=v_sb, in_=v[s].rearrange("(kc p) d -> p kc d", p=128))
        out_slab = pools["out"].tile([128, QT, D], F32, tag="outslab")
        if cfg["skip_pv"]:
            nc.vector.memset(out_slab[:, 0, :], 0.0)
        rs_slab = None
        if io16:
            rs_slab = pools["out"].tile([128, QT], F32, tag="rsslab")

        G = cfg["pv_group"]
        for qg in range(QT // G):
            # E^T staging for the group's q-tiles, fp16,
            # laid out [k_local(128 part), j(G), kc(16), q_local(128)]
            et_sb = pools["et"].tile([128, G, KC, 128], F16, tag="et")
            recips = []
            pos_pair_t = None
            if cfg["pos_pair"]:
                q0p = qg * G * 128
                pos_pair_t = pools["pos"].tile([128, G, S], e_dt if io16 else F32, tag="pos")
                nc.sync.dma_start(
                    out=pos_pair_t,
                    in_=pos[s, q0p : q0p + G * 128, :].rearrange(
                        "(j p) m -> p j m", p=128
                    ),
                )
            for j in range(G):
                qt = qg * G + j
                q0 = qt * 128
                if cfg["pos_pair"]:
                    pos_t = pos_pair_t[:, j, :]
                else:
                    pos_t = pools["pos"].tile([128, S], e_dt if io16 else F32, tag="pos")
                    nc.sync.dma_start(out=pos_t, in_=pos[s, q0 : q0 + 128, :])

                e_sb = pools["e"].tile([128, S], e_dt, tag="e")
                pe_halves = cfg["posadd_pe"] // 2 if io16 else 0
                half_sums = []
                for h in range(2):
                    h0 = h * 1024
                    sc_ps = pools["ps_sc"].tile([128, 2, 512], F32, tag="sc")
                    on_pe = h < pe_halves
                    for cc in range(2):
                        ks = h0 + cc * 512
                        nc.tensor.matmul(
                            sc_ps[:, cc, :],
                            lhsT=qT_sb[:, q0 : q0 + 128],
                            rhs=kT_sb[:, ks : ks + 512],
                            start=True,
                            stop=not on_pe,
                        )
                        if on_pe:
                            # add pos via identity matmul (exact: fp16->f32)
                            nc.tensor.matmul(
                                sc_ps[:, cc, :],
                                lhsT=ident16,
                                rhs=pos_t[:, ks : ks + 512],
                                start=False,
                                stop=True,
                            )
                    if on_pe:
                        exp_src = sc_ps.rearrange("p a b -> p (a b)")
                    else:
                        s_half = pools["s"].tile([128, 1024], F32, tag="s")
                        for cc in range(2):
                            ks = h0 + cc * 512
                            nc.vector.tensor_add(
                                s_half[:, cc * 512 : (cc + 1) * 512],
                                sc_ps[:, cc, :],
                                pos_t[:, ks : ks + 512],
                            )
                        exp_src = s_half
                    hs = pools["stat"].tile([128, 1], F32, tag=f"hs{h}")
                    nc.scalar.activation(
                        e_sb[:, h0 : h0 + 1024],
                        exp_src,
                        mybir.ActivationFunctionType.Exp,
                        bias=0.0,
                        scale=SCALE,
                        accum_out=hs,
                    )
                    half_sums.append(hs)
                rowsum = (
                    rs_slab[:, qt : qt + 1]
                    if io16
                    else pools["stat"].tile([128, 1], F32, tag="rowsum")
                )
                nc.vector.tensor_add(rowsum, half_sums[0], half_sums[1])
                recip = pools["stat"].tile([128, 1], F32, tag="recip")
                nc.vector.reciprocal(recip, rowsum)
                recips.append(recip)

                if io16:
                    # store unnormalized E (fp16); host divides by rowsum
                    if not cfg["skip_store"]:
                        store_eng.dma_start(out=p[s, q0 : q0 + 128, :], in_=e_sb)
                    if cfg["skip_pv"]:
                        continue
                    # transpose E 128x128 fp16 tiles via PE, 8 per PSUM bank
                    for c in range(2):
                        et_ps = pools["ps_et"].tile([128, 8, 128], F16, tag="etps")
                        for jj in range(8):
                            kc = c * 8 + jj
                            nc.tensor.transpose(
                                et_ps[:, jj, :],
                                e_sb[:, kc * 128 : (kc + 1) * 128],
                                ident16,
                            )
                        dst = et_sb[:, j, c * 8 : (c + 1) * 8, :]
                        if cfg["etcopy"] == "vector" or (
                            cfg["etcopy"] == "split" and c == 1
                        ):
                            nc.vector.tensor_copy(dst, et_ps)
                        else:
                            nc.scalar.copy(dst, et_ps)
                else:
                    p_sb = pools["p"].tile([128, S], F32, tag="p")
                    nc.vector.tensor_scalar_mul(p_sb, e_sb, recip)
                    store_eng.dma_start(out=p[s, q0 : q0 + 128, :], in_=p_sb)

                    # transpose E 128x128 tiles via PE, 4 per PSUM bank,
                    # then one ACT copy-cast f32->fp16 per bank
                    for c in range(4):
                        et_ps = pools["ps_et"].tile([128, 4, 128], F32, tag="etps")
                        for jj in range(4):
                            kc = c * 4 + jj
                            nc.tensor.transpose(
                                et_ps[:, jj, :],
                                e_sb[:, kc * 128 : (kc + 1) * 128],
                                ident,
                            )
                        nc.scalar.copy(et_sb[:, j, c * 4 : (c + 1) * 4, :], et_ps)

            if cfg["skip_pv"]:
                continue
            # PV for the group: out^T[d, (j, q_local)] accumulated over kc
            ot_ps = pools["ps_ot"].tile([D, G, 128], F32, tag="ot")
            for kc in range(KC):
                nc.tensor.matmul(
                    ot_ps,
                    lhsT=v_sb[:, kc, :],
                    rhs=et_sb[:, :, kc, :],
                    start=(kc == 0),
                    stop=(kc == KC - 1),
                )
            ot_sb = pools["stat"].tile([D, G, 128], F32, tag="ot_sb")
            if cfg["otfix_act"]:
                nc.scalar.copy(ot_sb, ot_ps)
            else:
                nc.vector.tensor_copy(ot_sb, ot_ps)
            for j in range(G):
                qt = qg * G + j
                of_ps = pools["ps_of"].tile([128, D], F32, tag="of")
                nc.tensor.transpose(of_ps, ot_sb[:, j, :], ident[:D, :D])
                if cfg["otfix_act"]:
                    nc.scalar.mul(out_slab[:, qt, :], of_ps, recips[j])
                else:
                    nc.vector.tensor_scalar_mul(out_slab[:, qt, :], of_ps, recips[j])

        nc.sync.dma_start(
            out=o[s].rearrange("(qt p) d -> p qt d", p=128), in_=out_slab
        )
        if io16:
            nc.sync.dma_start(
                out=rs[s].rearrange("(qt p) -> p qt", p=128), in_=rs_slab
            )


_NC = None


def _get_program():
    global _NC
    if _NC is None:
        _NC = _build_program()
    return _NC


def _prep_inputs(query, key, value, pos_attn, mask):
    """Host-side shard + pack: per-core input maps."""
    query = np.asarray(query, dtype=np.float32)
    key_ = np.asarray(key, dtype=np.float32)
    value = np.asarray(value, dtype=np.float32)
    pos_attn = np.asarray(pos_attn)
    mask = np.asarray(mask)

    # madd[b, k]: 0 where mask==1 else MASK_BIG (exact in fp16)
    madd = np.where(mask[:, 0, 0, :] == 0, np.float32(MASK_BIG), np.float32(0.0))

    pos_dt = np.float16 if OPT["io16"] else np.float32
    in_maps = []
    for core in range(N_CORES):
        qT_arr = np.zeros((SLABS_PER_CORE, CROWS, S), dtype=np.float16)
        kT_arr = np.zeros((SLABS_PER_CORE, CROWS, S), dtype=np.float16)
        v_arr = np.zeros((SLABS_PER_CORE, S, D), dtype=np.float16)
        pos_arr = np.empty((SLABS_PER_CORE, S, S), dtype=pos_dt)
        for s in range(SLABS_PER_CORE):
            slab = core * SLABS_PER_CORE + s
            b, h = slab // H, slab % H
            qT_arr[s, :D, :] = query[b, h].T.astype(np.float16)
            qT_arr[s, D, :] = np.float16(1.0)
            kT_arr[s, :D, :] = key_[b, h].T.astype(np.float16)
            kT_arr[s, D, :] = madd[b].astype(np.float16)
            v_arr[s] = value[b, h].astype(np.float16)
            pos_arr[s] = np.asarray(pos_attn[b, h], dtype=pos_dt)
        in_maps.append({"qT": qT_arr, "kT": kT_arr, "v": v_arr, "pos": pos_arr})
    return in_maps


LAST_RESULTS = None


def kernel(query, key, value, pos_attn, mask, **run_kwargs):
    global LAST_RESULTS
    nc = _get_program()
    in_maps = _prep_inputs(query, key, value, pos_attn, mask)
    res = run_bass_kernel_spmd(
        nc, in_maps, core_ids=list(range(N_CORES)), **run_kwargs
    )
    LAST_RESULTS = res

    out = np.empty((B, H, S, D), dtype=np.float32)
    p_attn = np.empty((B, H, S, S), dtype=np.float32)
    for core in range(N_CORES):
        rm = res.results[core]
        for s in range(SLABS_PER_CORE):
            slab = core * SLABS_PER_CORE + s
            b, h = slab // H, slab % H
            out[b, h] = rm["o"][s]
            if OPT["io16"]:
                recip = (1.0 / rm["rs"][s]).astype(np.float32)
                np.multiply(rm["p"][s], recip[:, None], out=p_attn[b, h])
            else:
                p_attn[b, h] = rm["p"][s]
    return out, p_attn
